# revision 10
# baseline (speedup 1.0000x reference)
"""Bahdanau-style attention kernel for Trainium2, data-parallel over batch
across 8 NeuronCores.

Reference computation (per batch b):
    e_proj = enc[b] @ We.T            # [S, D]   (We = W[:, 512:], [D, E])
    energy = tanh(e_proj + hidden[b] @ Wh.T + bias)
    scores = energy @ v               # [S]
    attn   = softmax(scores)          # [1, S]

Shapes: B=32, S=2048, E=1024, D=512.  Each core handles 4 batches.

Device-side layout (per core, orientation "B" = output transposed [d, s]):
  - enc tiles are transposed on TensorE (identity matmul) so the
    contraction dim e lands on partitions.
  - main matmul: psum[d128, s512] += WeT[e128, d128].T @ encT[e128, s512]
    in float32r (fp32 data rounded by the producing copies; full-rate on
    the PE for N>=256).
  - tanh fused with the (h_proj + b) bias via ScalarE activation
    (per-partition bias, since d is the partition dim).
  - scores via TensorE matvec with v; softmax on a [16, 128] layout.
"""

import numpy as np

B, S, E, D = 32, 2048, 1024, 512
N_CORES = 8
BP = B // N_CORES  # batches per core = 4
SBLK = 512  # s-block (psum free dim)
N_SBLK = S // SBLK  # 4
N_ST = SBLK // 128  # 4 s-subtiles per block
N_EC = E // 128  # 8 e-chunks
N_DP = D // 128  # 4 d-chunks
N_KC = D // 128  # 4 k-chunks (hidden proj contraction)

_CACHE = {}


def _build(debug_dumps=False):
    from contextlib import ExitStack

    import concourse.bass as bass
    import concourse.tile as tile
    from concourse import bacc, mybir
    from concourse.masks import make_identity

    F32 = mybir.dt.float32
    F32R = mybir.dt.float32r
    AF = mybir.ActivationFunctionType
    ALU = mybir.AluOpType
    AX = mybir.AxisListType

    nc = bacc.Bacc("TRN2", target_bir_lowering=False, debug=False,
                   num_devices=N_CORES)

    hid_d = nc.dram_tensor("hidden", [BP, D], F32, kind="ExternalInput").ap()
    enc_d = nc.dram_tensor("enc", [BP, S, E], F32, kind="ExternalInput").ap()
    w_d = nc.dram_tensor("W", [D, D + E], F32, kind="ExternalInput").ap()
    b_d = nc.dram_tensor("b", [D], F32, kind="ExternalInput").ap()
    v_d = nc.dram_tensor("v", [D], F32, kind="ExternalInput").ap()
    out_d = nc.dram_tensor("out", [BP, S], F32, kind="ExternalOutput").ap()
    if debug_dumps:
        dbg_scores = nc.dram_tensor(
            "dbg_scores", [BP, S], F32, kind="ExternalOutput").ap()
        dbg_energy = nc.dram_tensor(
            "dbg_energy", [128, N_DP, SBLK], F32, kind="ExternalOutput").ap()
        dbg_enct = nc.dram_tensor(
            "dbg_enct", [128, N_EC, SBLK], F32, kind="ExternalOutput").ap()
        dbg_hbt = nc.dram_tensor(
            "dbg_hbt", [128, N_DP, BP], F32, kind="ExternalOutput").ap()
        dbg_wet = nc.dram_tensor(
            "dbg_wet", [128, N_EC, D], F32, kind="ExternalOutput").ap()

    with tile.TileContext(nc) as tc, ExitStack() as ctx:
        consts = ctx.enter_context(tc.tile_pool(name="consts", bufs=1))
        enc_pool = ctx.enter_context(tc.tile_pool(name="enc", bufs=3))
        work = ctx.enter_context(tc.tile_pool(name="work", bufs=2))
        small = ctx.enter_context(tc.tile_pool(name="small", bufs=2))
        ps = ctx.enter_context(tc.tile_pool(name="ps", bufs=2, space="PSUM"))
        ps2 = ctx.enter_context(tc.tile_pool(name="ps2", bufs=2, space="PSUM"))
        drp = ctx.enter_context(tc.tile_pool(name="drp", bufs=2, space="DRAM"))

        identity = consts.tile([128, 128], F32)
        make_identity(nc, identity)
        ones16 = consts.tile([1, 16], F32)
        nc.vector.memset(ones16, 1.0)

        # ---- load weights & small inputs ----
        w_sb = consts.tile([128, N_DP, D + E], F32)
        nc.sync.dma_start(out=w_sb, in_=w_d.rearrange("(dp p) q -> p dp q", p=128))
        hid_sb = consts.tile([BP, D], F32)
        nc.sync.dma_start(out=hid_sb, in_=hid_d)
        b_sb4 = consts.tile([N_DP, 128], F32)
        nc.sync.dma_start(out=b_sb4, in_=b_d.rearrange("(dp q) -> dp q", q=128))
        v_sb4 = consts.tile([N_DP, 128], F32)
        nc.sync.dma_start(out=v_sb4, in_=v_d.rearrange("(dp q) -> dp q", q=128))

        # preload the exp/tanh activation table early (overlaps with DMAs)
        warm = consts.tile([1, 1], F32)
        nc.vector.memset(warm, 0.0)
        nc.scalar.activation(warm, warm, AF.Tanh)

        # ---- transpose We -> WeT [e, d] (fp32r), Wh -> WhT [k, d] ----
        wet_sb = consts.tile([128, N_EC, D], F32R)
        for ec in range(N_EC):
            pt = ps.tile([128, 512], F32, tag="ptr")
            with tc.tile_critical():
                for dp in range(N_DP):
                    nc.tensor.matmul(
                        pt[:, dp * 128:(dp + 1) * 128],
                        w_sb[:, dp, D + ec * 128: D + (ec + 1) * 128],
                        identity, is_transpose=True,
                        start=(dp == 0), stop=(dp == N_DP - 1),
                    )
            nc.vector.tensor_copy(wet_sb[:, ec, :], pt)

        wht_sb = consts.tile([128, N_KC, D], F32)
        for kc in range(N_KC):
            pt = ps.tile([128, 512], F32, tag="ptr")
            with tc.tile_critical():
                for dp in range(N_DP):
                    nc.tensor.matmul(
                        pt[:, dp * 128:(dp + 1) * 128],
                        w_sb[:, dp, kc * 128:(kc + 1) * 128],
                        identity, is_transpose=True,
                        start=(dp == 0), stop=(dp == N_DP - 1),
                    )
            nc.scalar.copy(wht_sb[:, kc, :], pt)

        # ---- hidden^T [k, b] ----
        hidt_sb = consts.tile([128, N_KC, BP], F32)
        for kc in range(N_KC):
            pt = ps2.tile([128, 16], F32, tag="sc")
            nc.tensor.transpose(
                pt[:, 0:BP], hid_sb[:, kc * 128:(kc + 1) * 128],
                identity[0:BP, 0:BP],
            )
            nc.vector.tensor_copy(hidt_sb[:, kc, :], pt[:, 0:BP])

        # ---- b^T, v^T  [128, dp] ----
        bt_sb = consts.tile([128, N_DP], F32)
        pt = ps2.tile([128, 16], F32, tag="sc")
        nc.tensor.transpose(pt[:, 0:N_DP], b_sb4, identity[0:N_DP, 0:N_DP])
        nc.vector.tensor_copy(bt_sb, pt[:, 0:N_DP])

        vt_sb = consts.tile([128, N_DP], F32R)
        pt = ps2.tile([128, 16], F32, tag="sc")
        nc.tensor.transpose(pt[:, 0:N_DP], v_sb4, identity[0:N_DP, 0:N_DP])
        nc.vector.tensor_copy(vt_sb, pt[:, 0:N_DP])

        # ---- h_projT + bias -> hbT [128, dp, b] ----
        hbt_sb = consts.tile([128, N_DP, BP], F32)
        for dp in range(N_DP):
            ph = ps2.tile([128, 16], F32, tag="sc")
            for kc in range(N_KC):
                nc.tensor.matmul(
                    ph[:, 0:BP],
                    wht_sb[:, kc, dp * 128:(dp + 1) * 128],
                    hidt_sb[:, kc, :],
                    start=(kc == 0), stop=(kc == N_KC - 1),
                )
            nc.vector.tensor_scalar_add(
                hbt_sb[:, dp, :], ph[:, 0:BP], bt_sb[:, dp:dp + 1]
            )

        # ---- main loop ----
        for bi in range(BP):
            scores_sb = small.tile([1, S], F32, tag="scores")
            for sblk in range(N_SBLK):
                enc_nat = enc_pool.tile([128, N_ST, E], F32, tag="enc_nat")
                nc.sync.dma_start(
                    out=enc_nat,
                    in_=enc_d[bi, sblk * SBLK:(sblk + 1) * SBLK, :].rearrange(
                        "(st p) e -> p st e", p=128
                    ),
                )

                enct_sb = work.tile([128, N_EC, SBLK], F32R, tag="encT")
                for ec in range(N_EC):
                    pt = ps.tile([128, 512], F32, tag="ptr")
                    with tc.tile_critical():
                        for st in range(N_ST):
                            nc.tensor.matmul(
                                pt[:, st * 128:(st + 1) * 128],
                                enc_nat[:, st, ec * 128:(ec + 1) * 128],
                                identity, is_transpose=True,
                                start=(st == 0), stop=(st == N_ST - 1),
                            )
                    if ec % 2 == 0:
                        nc.vector.tensor_copy(enct_sb[:, ec, :], pt)
                    else:
                        nc.scalar.copy(enct_sb[:, ec, :], pt)

                energy_sb = work.tile([128, N_DP, SBLK], F32R, tag="energy")
                for dp in range(N_DP):
                    pe = ps.tile([128, SBLK], F32, tag="pe")
                    for ec in range(N_EC):
                        nc.tensor.matmul(
                            pe,
                            wet_sb[:, ec, dp * 128:(dp + 1) * 128],
                            enct_sb[:, ec, :],
                            start=(ec == 0), stop=(ec == N_EC - 1),
                        )
                    nc.scalar.activation(
                        energy_sb[:, dp, :], pe, AF.Tanh,
                        bias=hbt_sb[:, dp, bi:bi + 1], scale=1.0,
                    )

                psc = ps2.tile([1, SBLK], F32, tag="vdot")
                for dp in range(N_DP):
                    nc.tensor.matmul(
                        psc, vt_sb[:, dp:dp + 1], energy_sb[:, dp, :],
                        start=(dp == 0), stop=(dp == N_DP - 1),
                    )
                nc.scalar.copy(scores_sb[:, sblk * SBLK:(sblk + 1) * SBLK], psc)

                if debug_dumps and bi == 0 and sblk == 0:
                    nc.sync.dma_start(out=dbg_enct, in_=enct_sb.bitcast(F32))
                    nc.sync.dma_start(out=dbg_energy, in_=energy_sb.bitcast(F32))

            # ---- softmax over S=2048 for this batch ----
            # SBUF APs cannot fold a free dim into partitions; bounce the
            # [1, S] scores row through DRAM to respread as [16, 128].
            scores_dr = drp.tile([1, S], F32, tag="scdr")
            nc.sync.dma_start(out=scores_dr, in_=scores_sb)
            sc128 = small.tile([16, 128], F32, tag="sc128")
            nc.sync.dma_start(
                out=sc128, in_=scores_dr.rearrange("o (j f) -> (o j) f", f=128)
            )
            m16 = small.tile([16, 1], F32, tag="m16")
            nc.vector.reduce_max(m16, sc128, axis=AX.X)
            ptm = ps2.tile([1, 16], F32, tag="sc")
            nc.tensor.transpose(ptm, m16, identity[0:16, 0:16])
            negm = small.tile([1, 16], F32, tag="negm")
            nc.scalar.mul(negm, ptm, -1.0)
            negm1 = small.tile([1, 1], F32, tag="negm1")
            nc.vector.tensor_reduce(negm1, negm, axis=AX.X, op=ALU.min)
            pbc = ps2.tile([16, 1], F32, tag="sc")
            nc.tensor.matmul(pbc, ones16, negm1, start=True, stop=True)
            negm16 = small.tile([16, 1], F32, tag="negm16")
            nc.scalar.copy(negm16, pbc)

            prob16 = small.tile([16, 128], F32, tag="prob16")
            nc.scalar.activation(prob16, sc128, AF.Exp, bias=negm16, scale=1.0)
            s16 = small.tile([16, 1], F32, tag="s16")
            nc.vector.reduce_sum(s16, prob16, axis=AX.X)
            pts = ps2.tile([1, 16], F32, tag="sc")
            nc.tensor.transpose(pts, s16, identity[0:16, 0:16])
            ssum = small.tile([1, 16], F32, tag="ssum")
            nc.vector.tensor_copy(ssum, pts)
            stot = small.tile([1, 1], F32, tag="stot")
            nc.vector.reduce_sum(stot, ssum, axis=AX.X)
            rtot = small.tile([1, 1], F32, tag="rtot")
            nc.vector.reciprocal(rtot, stot)
            pbc2 = ps2.tile([16, 1], F32, tag="sc")
            nc.tensor.matmul(pbc2, ones16, rtot, start=True, stop=True)
            r16 = small.tile([16, 1], F32, tag="r16")
            nc.scalar.copy(r16, pbc2)

            attn16 = small.tile([16, 128], F32, tag="attn16")
            nc.vector.tensor_scalar_mul(attn16, prob16, r16)
            nc.sync.dma_start(
                out=out_d[bi].rearrange("(j f) -> j f", f=128), in_=attn16
            )
            if debug_dumps:
                nc.sync.dma_start(out=dbg_scores[bi:bi + 1, :], in_=scores_sb)

        if debug_dumps:
            nc.sync.dma_start(out=dbg_hbt, in_=hbt_sb)
            nc.sync.dma_start(out=dbg_wet, in_=wet_sb.bitcast(F32))

    nc.compile()
    return nc


def _get_nc():
    if "nc" not in _CACHE:
        _CACHE["nc"] = _build()
    return _CACHE["nc"]


def kernel(hidden, encoder_outputs, W, b, v):
    from concourse.bass_utils import run_bass_kernel_spmd

    nc = _get_nc()
    hidden = np.ascontiguousarray(hidden, dtype=np.float32)
    encoder_outputs = np.ascontiguousarray(encoder_outputs, dtype=np.float32)
    W = np.ascontiguousarray(W, dtype=np.float32)
    b = np.ascontiguousarray(b, dtype=np.float32)
    v = np.ascontiguousarray(v, dtype=np.float32)

    in_maps = [
        {
            "hidden": hidden[c * BP:(c + 1) * BP],
            "enc": encoder_outputs[c * BP:(c + 1) * BP],
            "W": W,
            "b": b,
            "v": v,
        }
        for c in range(N_CORES)
    ]
    r = run_bass_kernel_spmd(nc, in_maps, list(range(N_CORES)))
    out = np.concatenate([r.results[c]["out"] for c in range(N_CORES)], axis=0)
    return out[:, None, :].astype(np.float32)


# revision 15
# speedup vs baseline: 1.1962x; 1.1962x over previous
"""Bahdanau-style attention kernel for Trainium2, data-parallel over batch
across 8 NeuronCores.

Reference computation (per batch b):
    e_proj = enc[b] @ We.T            # [S, D]   (We = W[:, 512:], [D, E])
    energy = tanh(e_proj + hidden[b] @ Wh.T + bias)
    scores = energy @ v               # [S]
    attn   = softmax(scores)          # [1, S]

Shapes: B=32, S=2048, E=1024, D=512.  Each core handles 4 batches.

Device-side design (per core):
  - enc is cast f32 -> fp16 during the DMA load (SWDGE), then transposed
    on TensorE so the contraction dim e lands on partitions.  fp16 keeps
    10 mantissa bits (end-to-end attn error ~1.5e-3) while streaming the
    PE at full rate with fast weight loads.
  - main matmul: psum[d128, s512] += WeT[e128, d128].T @ encT[e128, s512]
  - tanh fused with the (h_proj + b) bias via ScalarE activation
    (per-partition bias, since d is the partition dim).
  - scores via TensorE matvec with v; softmax on a [16, 128] layout
    (scores respread through a DRAM bounce - SBUF APs cannot fold a free
    dim into partitions).
"""

import numpy as np

B, S, E, D = 32, 2048, 1024, 512
N_CORES = 8
BP = B // N_CORES  # batches per core = 4
SBLK = 512  # s-block (psum free dim)
N_SBLK = S // SBLK  # 4
N_ST = SBLK // 128  # 4 s-subtiles per block
N_EC = E // 128  # 8 e-chunks
N_DP = D // 128  # 4 d-chunks
N_KC = D // 128  # 4 k-chunks (hidden proj contraction)

_CACHE = {}


def _build(debug_dumps=False):
    from contextlib import ExitStack

    import concourse.bass as bass
    import concourse.tile as tile
    from concourse import bacc, mybir
    from concourse.masks import make_identity

    F32 = mybir.dt.float32
    F16 = mybir.dt.float16
    AF = mybir.ActivationFunctionType
    ALU = mybir.AluOpType
    AX = mybir.AxisListType

    nc = bacc.Bacc("TRN2", target_bir_lowering=False, debug=False,
                   num_devices=N_CORES)

    hid_d = nc.dram_tensor("hidden", [BP, D], F32, kind="ExternalInput").ap()
    enc_d = nc.dram_tensor("enc", [BP, S, E], F32, kind="ExternalInput").ap()
    w_d = nc.dram_tensor("W", [D, D + E], F32, kind="ExternalInput").ap()
    b_d = nc.dram_tensor("b", [D], F32, kind="ExternalInput").ap()
    v_d = nc.dram_tensor("v", [D], F32, kind="ExternalInput").ap()
    out_d = nc.dram_tensor("out", [BP, S], F32, kind="ExternalOutput").ap()
    if debug_dumps:
        dbg_scores = nc.dram_tensor(
            "dbg_scores", [BP, S], F32, kind="ExternalOutput").ap()
        dbg_energy = nc.dram_tensor(
            "dbg_energy", [128, N_DP, SBLK], F16, kind="ExternalOutput").ap()
        dbg_enct = nc.dram_tensor(
            "dbg_enct", [128, N_EC, S], F16, kind="ExternalOutput").ap()
        dbg_hbt = nc.dram_tensor(
            "dbg_hbt", [128, N_DP, BP], F32, kind="ExternalOutput").ap()
        dbg_wet = nc.dram_tensor(
            "dbg_wet", [128, N_EC, D], F16, kind="ExternalOutput").ap()

    with tile.TileContext(nc) as tc, ExitStack() as ctx:
        consts = ctx.enter_context(tc.tile_pool(name="consts", bufs=1))
        enc_pool = ctx.enter_context(tc.tile_pool(name="enc", bufs=6))
        work = ctx.enter_context(tc.tile_pool(name="work", bufs=2))
        small = ctx.enter_context(tc.tile_pool(name="small", bufs=2))
        ps = ctx.enter_context(tc.tile_pool(name="ps", bufs=2, space="PSUM"))
        ps2 = ctx.enter_context(tc.tile_pool(name="ps2", bufs=2, space="PSUM"))
        drp = ctx.enter_context(tc.tile_pool(name="drp", bufs=2, space="DRAM"))

        identity = consts.tile([128, 128], F32)
        make_identity(nc, identity)
        identity16 = consts.tile([128, 128], F16)
        make_identity(nc, identity16)
        ones16 = consts.tile([1, 16], F32)
        nc.vector.memset(ones16, 1.0)

        # ---- load weights & small inputs ----
        w_sb = consts.tile([128, N_DP, D + E], F32)
        nc.sync.dma_start(out=w_sb, in_=w_d.rearrange("(dp p) q -> p dp q", p=128))
        hid_sb = consts.tile([BP, D], F32)
        nc.sync.dma_start(out=hid_sb, in_=hid_d)
        b_sb4 = consts.tile([N_DP, 128], F32)
        nc.sync.dma_start(out=b_sb4, in_=b_d.rearrange("(dp q) -> dp q", q=128))
        v_sb4 = consts.tile([N_DP, 128], F32)
        nc.sync.dma_start(out=v_sb4, in_=v_d.rearrange("(dp q) -> dp q", q=128))

        # preload the exp/tanh activation table early (overlaps with DMAs)
        warm = consts.tile([1, 1], F32)
        nc.vector.memset(warm, 0.0)
        nc.scalar.activation(warm, warm, AF.Tanh)

        # ---- transpose We -> WeT [e, d] (fp16), Wh -> WhT [k, d] ----
        wet_sb = consts.tile([128, N_EC, D], F16)
        for ec in range(N_EC):
            pt = ps.tile([128, 512], F32, tag="ptr")
            with tc.tile_critical():
                for dp in range(N_DP):
                    nc.tensor.matmul(
                        pt[:, dp * 128:(dp + 1) * 128],
                        w_sb[:, dp, D + ec * 128: D + (ec + 1) * 128],
                        identity, is_transpose=True,
                        start=(dp == 0), stop=(dp == N_DP - 1),
                    )
            nc.vector.tensor_copy(wet_sb[:, ec, :], pt)

        wht_sb = consts.tile([128, N_KC, D], F32)
        for kc in range(N_KC):
            pt = ps.tile([128, 512], F32, tag="ptr")
            with tc.tile_critical():
                for dp in range(N_DP):
                    nc.tensor.matmul(
                        pt[:, dp * 128:(dp + 1) * 128],
                        w_sb[:, dp, kc * 128:(kc + 1) * 128],
                        identity, is_transpose=True,
                        start=(dp == 0), stop=(dp == N_DP - 1),
                    )
            nc.scalar.copy(wht_sb[:, kc, :], pt)

        # ---- hidden^T [k, b] ----
        hidt_sb = consts.tile([128, N_KC, BP], F32)
        for kc in range(N_KC):
            pt = ps2.tile([128, 16], F32, tag="sc")
            nc.tensor.transpose(
                pt[:, 0:BP], hid_sb[:, kc * 128:(kc + 1) * 128],
                identity[0:BP, 0:BP],
            )
            nc.vector.tensor_copy(hidt_sb[:, kc, :], pt[:, 0:BP])

        # ---- b^T, v^T  [128, dp] ----
        bt_sb = consts.tile([128, N_DP], F32)
        pt = ps2.tile([128, 16], F32, tag="sc")
        nc.tensor.transpose(pt[:, 0:N_DP], b_sb4, identity[0:N_DP, 0:N_DP])
        nc.vector.tensor_copy(bt_sb, pt[:, 0:N_DP])

        vt_sb = consts.tile([128, N_DP], F16)
        pt = ps2.tile([128, 16], F32, tag="sc")
        nc.tensor.transpose(pt[:, 0:N_DP], v_sb4, identity[0:N_DP, 0:N_DP])
        nc.vector.tensor_copy(vt_sb, pt[:, 0:N_DP])

        # ---- h_projT + bias -> hbT [128, dp, b] ----
        hbt_sb = consts.tile([128, N_DP, BP], F32)
        for dp in range(N_DP):
            ph = ps2.tile([128, 16], F32, tag="sc")
            for kc in range(N_KC):
                nc.tensor.matmul(
                    ph[:, 0:BP],
                    wht_sb[:, kc, dp * 128:(dp + 1) * 128],
                    hidt_sb[:, kc, :],
                    start=(kc == 0), stop=(kc == N_KC - 1),
                )
            nc.vector.tensor_scalar_add(
                hbt_sb[:, dp, :], ph[:, 0:BP], bt_sb[:, dp:dp + 1]
            )

        # ---- main loop ----
        for bi in range(BP):
            # load the whole batch as fp16 (cast during SWDGE DMA)
            enc_nat = [None] * N_SBLK
            for sblk in range(N_SBLK):
                enc_tile = enc_pool.tile([128, N_ST, E], F16, tag="enc_nat")
                enc_nat[sblk] = enc_tile
                nc.gpsimd.dma_start(
                    out=enc_tile,
                    in_=enc_d[bi, sblk * SBLK:(sblk + 1) * SBLK, :].rearrange(
                        "(st p) e -> p st e", p=128
                    ),
                )

            # transpose the whole batch: encT[e, s] for s in [0, 2048)
            enct_sb = work.tile([128, N_EC, S], F16, tag="encT")
            for sblk in range(N_SBLK):
                for ech in range(N_EC // 2):
                    pt = ps.tile([128, 1024], F16, tag="ptr")
                    with tc.tile_critical():
                        for half in range(2):
                            ec = ech * 2 + half
                            for st in range(N_ST):
                                nc.tensor.matmul(
                                    pt[:, half * 512 + st * 128:
                                       half * 512 + (st + 1) * 128],
                                    enc_nat[sblk][:, st, ec * 128:(ec + 1) * 128],
                                    identity16, is_transpose=True,
                                    start=(half == 0 and st == 0),
                                    stop=(half == 1 and st == N_ST - 1),
                                )
                    for half in range(2):
                        ec = ech * 2 + half
                        dst = enct_sb[:, ec, sblk * SBLK:(sblk + 1) * SBLK]
                        if ec % 2 == 0:
                            nc.vector.tensor_copy(dst, pt[:, half * 512:(half + 1) * 512])
                        else:
                            nc.scalar.copy(dst, pt[:, half * 512:(half + 1) * 512])

            scores_sb = small.tile([1, S], F32, tag="scores")
            for sblk in range(N_SBLK):
                energy_sb = work.tile([128, N_DP, SBLK], F16, tag="energy")
                for dp in range(N_DP):
                    pe = ps.tile([128, SBLK], F32, tag="pe")
                    for ec in range(N_EC):
                        nc.tensor.matmul(
                            pe,
                            wet_sb[:, ec, dp * 128:(dp + 1) * 128],
                            enct_sb[:, ec, sblk * SBLK:(sblk + 1) * SBLK],
                            start=(ec == 0), stop=(ec == N_EC - 1),
                        )
                    nc.scalar.activation(
                        energy_sb[:, dp, :], pe, AF.Tanh,
                        bias=hbt_sb[:, dp, bi:bi + 1], scale=1.0,
                    )

                psc = ps2.tile([1, SBLK], F32, tag="vdot")
                for dp in range(N_DP):
                    nc.tensor.matmul(
                        psc, vt_sb[:, dp:dp + 1], energy_sb[:, dp, :],
                        start=(dp == 0), stop=(dp == N_DP - 1),
                    )
                nc.scalar.copy(scores_sb[:, sblk * SBLK:(sblk + 1) * SBLK], psc)

                if debug_dumps and bi == 0 and sblk == 0:
                    nc.sync.dma_start(out=dbg_energy, in_=energy_sb)

            if debug_dumps and bi == 0:
                nc.sync.dma_start(out=dbg_enct, in_=enct_sb)

            # ---- softmax over S=2048 for this batch ----
            # SBUF APs cannot fold a free dim into partitions; bounce the
            # [1, S] scores row through DRAM to respread as [16, 128].
            scores_dr = drp.tile([1, S], F32, tag="scdr")
            nc.sync.dma_start(out=scores_dr, in_=scores_sb)
            sc128 = small.tile([16, 128], F32, tag="sc128")
            nc.sync.dma_start(
                out=sc128, in_=scores_dr.rearrange("o (j f) -> (o j) f", f=128)
            )
            m16 = small.tile([16, 1], F32, tag="m16")
            nc.vector.reduce_max(m16, sc128, axis=AX.X)
            ptm = ps2.tile([1, 16], F32, tag="sc")
            nc.tensor.transpose(ptm, m16, identity[0:16, 0:16])
            negm = small.tile([1, 16], F32, tag="negm")
            nc.scalar.mul(negm, ptm, -1.0)
            negm1 = small.tile([1, 1], F32, tag="negm1")
            nc.vector.tensor_reduce(negm1, negm, axis=AX.X, op=ALU.min)
            pbc = ps2.tile([16, 1], F32, tag="sc")
            nc.tensor.matmul(pbc, ones16, negm1, start=True, stop=True)
            negm16 = small.tile([16, 1], F32, tag="negm16")
            nc.scalar.copy(negm16, pbc)

            prob16 = small.tile([16, 128], F32, tag="prob16")
            nc.scalar.activation(prob16, sc128, AF.Exp, bias=negm16, scale=1.0)
            s16 = small.tile([16, 1], F32, tag="s16")
            nc.vector.reduce_sum(s16, prob16, axis=AX.X)
            pts = ps2.tile([1, 16], F32, tag="sc")
            nc.tensor.transpose(pts, s16, identity[0:16, 0:16])
            ssum = small.tile([1, 16], F32, tag="ssum")
            nc.vector.tensor_copy(ssum, pts)
            stot = small.tile([1, 1], F32, tag="stot")
            nc.vector.reduce_sum(stot, ssum, axis=AX.X)
            rtot = small.tile([1, 1], F32, tag="rtot")
            nc.vector.reciprocal(rtot, stot)
            pbc2 = ps2.tile([16, 1], F32, tag="sc")
            nc.tensor.matmul(pbc2, ones16, rtot, start=True, stop=True)
            r16 = small.tile([16, 1], F32, tag="r16")
            nc.scalar.copy(r16, pbc2)

            attn16 = small.tile([16, 128], F32, tag="attn16")
            nc.vector.tensor_scalar_mul(attn16, prob16, r16)
            nc.sync.dma_start(
                out=out_d[bi].rearrange("(j f) -> j f", f=128), in_=attn16
            )
            if debug_dumps:
                nc.sync.dma_start(out=dbg_scores[bi:bi + 1, :], in_=scores_sb)

        if debug_dumps:
            nc.sync.dma_start(out=dbg_hbt, in_=hbt_sb)
            nc.sync.dma_start(out=dbg_wet, in_=wet_sb)

    nc.compile()
    return nc


def _get_nc():
    if "nc" not in _CACHE:
        _CACHE["nc"] = _build()
    return _CACHE["nc"]


def kernel(hidden, encoder_outputs, W, b, v):
    from concourse.bass_utils import run_bass_kernel_spmd

    nc = _get_nc()
    hidden = np.ascontiguousarray(hidden, dtype=np.float32)
    encoder_outputs = np.ascontiguousarray(encoder_outputs, dtype=np.float32)
    W = np.ascontiguousarray(W, dtype=np.float32)
    b = np.ascontiguousarray(b, dtype=np.float32)
    v = np.ascontiguousarray(v, dtype=np.float32)

    in_maps = [
        {
            "hidden": hidden[c * BP:(c + 1) * BP],
            "enc": encoder_outputs[c * BP:(c + 1) * BP],
            "W": W,
            "b": b,
            "v": v,
        }
        for c in range(N_CORES)
    ]
    r = run_bass_kernel_spmd(nc, in_maps, list(range(N_CORES)))
    out = np.concatenate([r.results[c]["out"] for c in range(N_CORES)], axis=0)
    return out[:, None, :].astype(np.float32)


# revision 18
# speedup vs baseline: 1.2071x; 1.0091x over previous
"""Bahdanau-style attention kernel for Trainium2, data-parallel over batch
across 8 NeuronCores.

Reference computation (per batch b):
    e_proj = enc[b] @ We.T            # [S, D]   (We = W[:, 512:], [D, E])
    energy = tanh(e_proj + hidden[b] @ Wh.T + bias)
    scores = energy @ v               # [S]
    attn   = softmax(scores)          # [1, S]

Shapes: B=32, S=2048, E=1024, D=512.  Each core handles 4 batches.

Device-side design (per core):
  - enc is cast f32 -> fp16 during the DMA load (SWDGE), then transposed
    on TensorE so the contraction dim e lands on partitions.  fp16 keeps
    10 mantissa bits (end-to-end attn error ~1.5e-3) while streaming the
    PE at full rate with fast weight loads.
  - main matmul: psum[d128, s512] += WeT[e128, d128].T @ encT[e128, s512]
  - tanh fused with the (h_proj + b) bias via ScalarE activation
    (per-partition bias, since d is the partition dim).
  - scores via TensorE matvec with v; softmax on a [16, 128] layout
    (scores respread through a DRAM bounce - SBUF APs cannot fold a free
    dim into partitions).
"""

import numpy as np

B, S, E, D = 32, 2048, 1024, 512
N_CORES = 8
BP = B // N_CORES  # batches per core = 4
SBLK = 512  # s-block (psum free dim)
N_SBLK = S // SBLK  # 4
N_ST = SBLK // 128  # 4 s-subtiles per block
N_EC = E // 128  # 8 e-chunks
N_DP = D // 128  # 4 d-chunks
N_KC = D // 128  # 4 k-chunks (hidden proj contraction)

_CACHE = {}


def _build(debug_dumps=False):
    from contextlib import ExitStack

    import concourse.bass as bass
    import concourse.tile as tile
    from concourse import bacc, mybir
    from concourse.masks import make_identity

    F32 = mybir.dt.float32
    F16 = mybir.dt.float16
    AF = mybir.ActivationFunctionType
    ALU = mybir.AluOpType
    AX = mybir.AxisListType

    nc = bacc.Bacc("TRN2", target_bir_lowering=False, debug=False,
                   num_devices=N_CORES)

    hid_d = nc.dram_tensor("hidden", [BP, D], F32, kind="ExternalInput").ap()
    enc_d = nc.dram_tensor("enc", [BP, S, E], F32, kind="ExternalInput").ap()
    w_d = nc.dram_tensor("W", [D, D + E], F32, kind="ExternalInput").ap()
    b_d = nc.dram_tensor("b", [D], F32, kind="ExternalInput").ap()
    v_d = nc.dram_tensor("v", [D], F32, kind="ExternalInput").ap()
    out_d = nc.dram_tensor("out", [BP, S], F32, kind="ExternalOutput").ap()
    if debug_dumps:
        dbg_scores = nc.dram_tensor(
            "dbg_scores", [BP, S], F32, kind="ExternalOutput").ap()
        dbg_energy = nc.dram_tensor(
            "dbg_energy", [128, N_DP, SBLK], F16, kind="ExternalOutput").ap()
        dbg_enct = nc.dram_tensor(
            "dbg_enct", [128, N_EC, S], F16, kind="ExternalOutput").ap()
        dbg_hbt = nc.dram_tensor(
            "dbg_hbt", [128, N_DP, BP], F32, kind="ExternalOutput").ap()
        dbg_wet = nc.dram_tensor(
            "dbg_wet", [128, N_EC, D], F16, kind="ExternalOutput").ap()

    with tile.TileContext(nc) as tc, ExitStack() as ctx:
        consts = ctx.enter_context(tc.tile_pool(name="consts", bufs=1))
        enc_pool = ctx.enter_context(tc.tile_pool(name="enc", bufs=5))
        work = ctx.enter_context(tc.tile_pool(name="work", bufs=2))
        small = ctx.enter_context(tc.tile_pool(name="small", bufs=2))
        sm1 = ctx.enter_context(tc.tile_pool(name="sm1", bufs=1))
        ps = ctx.enter_context(tc.tile_pool(name="ps", bufs=2, space="PSUM"))
        ps2 = ctx.enter_context(tc.tile_pool(name="ps2", bufs=2, space="PSUM"))

        identity = consts.tile([128, 128], F32)
        make_identity(nc, identity)
        identity16 = consts.tile([128, 128], F16)
        make_identity(nc, identity16)

        # ---- load weights & small inputs ----
        w_sb = work.tile([128, N_DP, D + E], F32, tag="encT")
        nc.sync.dma_start(out=w_sb, in_=w_d.rearrange("(dp p) q -> p dp q", p=128))
        hid_sb = consts.tile([BP, D], F32)
        nc.sync.dma_start(out=hid_sb, in_=hid_d)
        b_sb4 = consts.tile([N_DP, 128], F32)
        nc.sync.dma_start(out=b_sb4, in_=b_d.rearrange("(dp q) -> dp q", q=128))
        v_sb4 = consts.tile([N_DP, 128], F32)
        nc.sync.dma_start(out=v_sb4, in_=v_d.rearrange("(dp q) -> dp q", q=128))

        # preload the exp/tanh activation table early (overlaps with DMAs)
        warm = consts.tile([1, 1], F32)
        nc.vector.memset(warm, 0.0)
        nc.scalar.activation(warm, warm, AF.Tanh)

        # ---- transpose We -> WeT [e, d] (fp16), Wh -> WhT [k, d] ----
        wet_sb = consts.tile([128, N_EC, D], F16)
        for ec in range(N_EC):
            pt = ps.tile([128, 512], F32, tag="ptr")
            with tc.tile_critical():
                for dp in range(N_DP):
                    nc.tensor.matmul(
                        pt[:, dp * 128:(dp + 1) * 128],
                        w_sb[:, dp, D + ec * 128: D + (ec + 1) * 128],
                        identity, is_transpose=True,
                        start=(dp == 0), stop=(dp == N_DP - 1),
                    )
            nc.vector.tensor_copy(wet_sb[:, ec, :], pt)

        wht_sb = consts.tile([128, N_KC, D], F32)
        for kc in range(N_KC):
            pt = ps.tile([128, 512], F32, tag="ptr")
            with tc.tile_critical():
                for dp in range(N_DP):
                    nc.tensor.matmul(
                        pt[:, dp * 128:(dp + 1) * 128],
                        w_sb[:, dp, kc * 128:(kc + 1) * 128],
                        identity, is_transpose=True,
                        start=(dp == 0), stop=(dp == N_DP - 1),
                    )
            nc.scalar.copy(wht_sb[:, kc, :], pt)

        # ---- hidden^T [k, b] ----
        hidt_sb = consts.tile([128, N_KC, BP], F32)
        for kc in range(N_KC):
            pt = ps2.tile([128, 16], F32, tag="sc")
            nc.tensor.transpose(
                pt[:, 0:BP], hid_sb[:, kc * 128:(kc + 1) * 128],
                identity[0:BP, 0:BP],
            )
            nc.vector.tensor_copy(hidt_sb[:, kc, :], pt[:, 0:BP])

        # ---- b^T, v^T  [128, dp] ----
        bt_sb = consts.tile([128, N_DP], F32)
        pt = ps2.tile([128, 16], F32, tag="sc")
        nc.tensor.transpose(pt[:, 0:N_DP], b_sb4, identity[0:N_DP, 0:N_DP])
        nc.vector.tensor_copy(bt_sb, pt[:, 0:N_DP])

        vt_sb = consts.tile([128, N_DP], F16)
        pt = ps2.tile([128, 16], F32, tag="sc")
        nc.tensor.transpose(pt[:, 0:N_DP], v_sb4, identity[0:N_DP, 0:N_DP])
        nc.vector.tensor_copy(vt_sb, pt[:, 0:N_DP])

        # ---- h_projT + bias -> hbT [128, dp, b] ----
        hbt_sb = consts.tile([128, N_DP, BP], F32)
        for dp in range(N_DP):
            ph = ps2.tile([128, 16], F32, tag="sc")
            for kc in range(N_KC):
                nc.tensor.matmul(
                    ph[:, 0:BP],
                    wht_sb[:, kc, dp * 128:(dp + 1) * 128],
                    hidt_sb[:, kc, :],
                    start=(kc == 0), stop=(kc == N_KC - 1),
                )
            nc.vector.tensor_scalar_add(
                hbt_sb[:, dp, :], ph[:, 0:BP], bt_sb[:, dp:dp + 1]
            )

        # ---- main loop ----
        for bi in range(BP):
            # load the whole batch as fp16 (cast during SWDGE DMA)
            enc_nat = [None] * N_SBLK
            for sblk in range(N_SBLK):
                enc_tile = enc_pool.tile([128, N_ST, E], F16, tag="enc_nat")
                enc_nat[sblk] = enc_tile
                nc.gpsimd.dma_start(
                    out=enc_tile,
                    in_=enc_d[bi, sblk * SBLK:(sblk + 1) * SBLK, :].rearrange(
                        "(st p) e -> p st e", p=128
                    ),
                )

            # transpose the whole batch: encT[e, s] for s in [0, 2048)
            enct_sb = work.tile([128, N_EC, S], F16, tag="encT")
            for sblk in range(N_SBLK):
                for ech in range(N_EC // 2):
                    pt = ps.tile([128, 1024], F16, tag="ptr")
                    with tc.tile_critical():
                        for half in range(2):
                            ec = ech * 2 + half
                            for st in range(N_ST):
                                nc.tensor.matmul(
                                    pt[:, half * 512 + st * 128:
                                       half * 512 + (st + 1) * 128],
                                    enc_nat[sblk][:, st, ec * 128:(ec + 1) * 128],
                                    identity16, is_transpose=True,
                                    start=(half == 0 and st == 0),
                                    stop=(half == 1 and st == N_ST - 1),
                                )
                    for half in range(2):
                        ec = ech * 2 + half
                        dst = enct_sb[:, ec, sblk * SBLK:(sblk + 1) * SBLK]
                        if ec % 2 == 0:
                            nc.vector.tensor_copy(dst, pt[:, half * 512:(half + 1) * 512])
                        else:
                            nc.scalar.copy(dst, pt[:, half * 512:(half + 1) * 512])

            scores_sb = small.tile([1, S], F32, tag="scores")
            for sblk in range(N_SBLK):
                energy_sb = work.tile([128, N_DP, SBLK], F16, tag="energy")
                for dp in range(N_DP):
                    pe = ps.tile([128, SBLK], F32, tag="pe")
                    for ec in range(N_EC):
                        nc.tensor.matmul(
                            pe,
                            wet_sb[:, ec, dp * 128:(dp + 1) * 128],
                            enct_sb[:, ec, sblk * SBLK:(sblk + 1) * SBLK],
                            start=(ec == 0), stop=(ec == N_EC - 1),
                        )
                    nc.scalar.activation(
                        energy_sb[:, dp, :], pe, AF.Tanh,
                        bias=hbt_sb[:, dp, bi:bi + 1], scale=1.0,
                    )

                psc = ps2.tile([1, SBLK], F32, tag="vdot")
                for dp in range(N_DP):
                    nc.tensor.matmul(
                        psc, vt_sb[:, dp:dp + 1], energy_sb[:, dp, :],
                        start=(dp == 0), stop=(dp == N_DP - 1),
                    )
                nc.scalar.copy(scores_sb[:, sblk * SBLK:(sblk + 1) * SBLK], psc)

                if debug_dumps and bi == 0 and sblk == 0:
                    nc.sync.dma_start(out=dbg_energy, in_=energy_sb)

            if debug_dumps and bi == 0:
                nc.sync.dma_start(out=dbg_enct, in_=enct_sb)

            # ---- softmax over S=2048 for this batch ----
            # Single-partition softmax on VectorE/ScalarE only: keeps the
            # PE stream free of cross-batch stalls (no transposes, no
            # broadcasts, no DRAM bounce).  ~9us/batch, fully overlapped
            # with the next batch's PE work.
            m1 = small.tile([1, 1], F32, tag="m1")
            nc.vector.reduce_max(m1, scores_sb, axis=AX.X)
            negm = small.tile([1, 1], F32, tag="negm")
            nc.vector.tensor_scalar_mul(negm, m1, -1.0)
            prob = sm1.tile([1, S], F32, tag="prob")
            nc.scalar.activation(prob, scores_sb, AF.Exp, bias=negm, scale=1.0)
            ssum = small.tile([1, 1], F32, tag="ssum")
            nc.vector.reduce_sum(ssum, prob, axis=AX.X)
            rtot = small.tile([1, 1], F32, tag="rtot")
            nc.vector.reciprocal(rtot, ssum)
            attn = sm1.tile([1, S], F32, tag="attn")
            nc.vector.tensor_scalar_mul(attn, prob, rtot)
            nc.sync.dma_start(out=out_d[bi], in_=attn)
            if debug_dumps:
                nc.sync.dma_start(out=dbg_scores[bi:bi + 1, :], in_=scores_sb)

        if debug_dumps:
            nc.sync.dma_start(out=dbg_hbt, in_=hbt_sb)
            nc.sync.dma_start(out=dbg_wet, in_=wet_sb)

    nc.compile()
    return nc


def _get_nc():
    if "nc" not in _CACHE:
        _CACHE["nc"] = _build()
    return _CACHE["nc"]


def kernel(hidden, encoder_outputs, W, b, v):
    from concourse.bass_utils import run_bass_kernel_spmd

    nc = _get_nc()
    hidden = np.ascontiguousarray(hidden, dtype=np.float32)
    encoder_outputs = np.ascontiguousarray(encoder_outputs, dtype=np.float32)
    W = np.ascontiguousarray(W, dtype=np.float32)
    b = np.ascontiguousarray(b, dtype=np.float32)
    v = np.ascontiguousarray(v, dtype=np.float32)

    in_maps = [
        {
            "hidden": hidden[c * BP:(c + 1) * BP],
            "enc": encoder_outputs[c * BP:(c + 1) * BP],
            "W": W,
            "b": b,
            "v": v,
        }
        for c in range(N_CORES)
    ]
    r = run_bass_kernel_spmd(nc, in_maps, list(range(N_CORES)))
    out = np.concatenate([r.results[c]["out"] for c in range(N_CORES)], axis=0)
    return out[:, None, :].astype(np.float32)


# revision 20
# speedup vs baseline: 1.5006x; 1.2431x over previous
"""Bahdanau-style attention kernel for Trainium2, data-parallel over batch
across 8 NeuronCores.

Reference computation (per batch b):
    e_proj = enc[b] @ We.T            # [S, D]   (We = W[:, 512:], [D, E])
    energy = tanh(e_proj + hidden[b] @ Wh.T + bias)
    scores = energy @ v               # [S]
    attn   = softmax(scores)          # [1, S]

Shapes: B=32, S=2048, E=1024, D=512.  Each core handles 4 batches.

Device-side design (per core):
  - enc is cast f32 -> fp16 during the DMA load (SWDGE), then transposed
    on TensorE so the contraction dim e lands on partitions.  fp16 keeps
    10 mantissa bits (end-to-end attn error ~1.5e-3) while streaming the
    PE at full rate with fast weight loads.
  - main matmul: psum[d128, s512] += WeT[e128, d128].T @ encT[e128, s512]
  - tanh fused with the (h_proj + b) bias via ScalarE activation
    (per-partition bias, since d is the partition dim).
  - scores via TensorE matvec with v; softmax on a [16, 128] layout
    (scores respread through a DRAM bounce - SBUF APs cannot fold a free
    dim into partitions).
"""

import numpy as np

B, S, E, D = 32, 2048, 1024, 512
N_CORES = 8
BP = B // N_CORES  # batches per core = 4
SBLK = 512  # s-block (psum free dim)
N_SBLK = S // SBLK  # 4
N_ST = SBLK // 128  # 4 s-subtiles per block
N_EC = E // 128  # 8 e-chunks
N_DP = D // 128  # 4 d-chunks
N_KC = D // 128  # 4 k-chunks (hidden proj contraction)

_CACHE = {}


def _build(debug_dumps=False):
    from contextlib import ExitStack

    import concourse.bass as bass
    import concourse.tile as tile
    from concourse import bacc, mybir
    from concourse.masks import make_identity

    F32 = mybir.dt.float32
    F16 = mybir.dt.float16
    AF = mybir.ActivationFunctionType
    ALU = mybir.AluOpType
    AX = mybir.AxisListType

    nc = bacc.Bacc("TRN2", target_bir_lowering=False, debug=False,
                   num_devices=N_CORES)

    hid_d = nc.dram_tensor("hidden", [BP, D], F32, kind="ExternalInput").ap()
    enc_d = nc.dram_tensor("enc", [BP, S, E], F32, kind="ExternalInput").ap()
    w_d = nc.dram_tensor("W", [D, D + E], F32, kind="ExternalInput").ap()
    b_d = nc.dram_tensor("b", [D], F32, kind="ExternalInput").ap()
    v_d = nc.dram_tensor("v", [D], F32, kind="ExternalInput").ap()
    out_d = nc.dram_tensor("out", [BP, S], F32, kind="ExternalOutput").ap()
    if debug_dumps:
        dbg_scores = nc.dram_tensor(
            "dbg_scores", [BP, S], F32, kind="ExternalOutput").ap()
        dbg_energy = nc.dram_tensor(
            "dbg_energy", [128, N_DP, SBLK], F16, kind="ExternalOutput").ap()
        dbg_enct = nc.dram_tensor(
            "dbg_enct", [128, N_EC, S], F16, kind="ExternalOutput").ap()
        dbg_hbt = nc.dram_tensor(
            "dbg_hbt", [128, N_DP, BP], F32, kind="ExternalOutput").ap()
        dbg_wet = nc.dram_tensor(
            "dbg_wet", [128, N_EC, D], F16, kind="ExternalOutput").ap()

    with tile.TileContext(nc) as tc, ExitStack() as ctx:
        consts = ctx.enter_context(tc.tile_pool(name="consts", bufs=1))
        enc_pool = ctx.enter_context(tc.tile_pool(name="enc", bufs=2))
        enc16_pool = ctx.enter_context(tc.tile_pool(name="enc16", bufs=3))
        work = ctx.enter_context(tc.tile_pool(name="work", bufs=2))
        small = ctx.enter_context(tc.tile_pool(name="small", bufs=2))
        sm1 = ctx.enter_context(tc.tile_pool(name="sm1", bufs=1))
        ps = ctx.enter_context(tc.tile_pool(name="ps", bufs=2, space="PSUM"))
        ps2 = ctx.enter_context(tc.tile_pool(name="ps2", bufs=2, space="PSUM"))

        identity = consts.tile([128, 128], F32)
        make_identity(nc, identity)
        identity16 = consts.tile([128, 128], F16)
        make_identity(nc, identity16)

        # ---- load weights & small inputs ----
        w_sb = work.tile([128, N_DP, D + E], F32, tag="encT")
        nc.sync.dma_start(out=w_sb, in_=w_d.rearrange("(dp p) q -> p dp q", p=128))
        hid_sb = consts.tile([BP, D], F32)
        nc.sync.dma_start(out=hid_sb, in_=hid_d)
        b_sb4 = consts.tile([N_DP, 128], F32)
        nc.sync.dma_start(out=b_sb4, in_=b_d.rearrange("(dp q) -> dp q", q=128))
        v_sb4 = consts.tile([N_DP, 128], F32)
        nc.sync.dma_start(out=v_sb4, in_=v_d.rearrange("(dp q) -> dp q", q=128))

        # preload the exp/tanh activation table early (overlaps with DMAs)
        warm = consts.tile([1, 1], F32)
        nc.vector.memset(warm, 0.0)
        nc.scalar.activation(warm, warm, AF.Tanh)

        # ---- transpose We -> WeT [e, d] (fp16), Wh -> WhT [k, d] ----
        wet_sb = consts.tile([128, N_EC, D], F16)
        for ec in range(N_EC):
            pt = ps.tile([128, 512], F32, tag="ptr")
            with tc.tile_critical():
                for dp in range(N_DP):
                    nc.tensor.matmul(
                        pt[:, dp * 128:(dp + 1) * 128],
                        w_sb[:, dp, D + ec * 128: D + (ec + 1) * 128],
                        identity, is_transpose=True,
                        start=(dp == 0), stop=(dp == N_DP - 1),
                    )
            nc.vector.tensor_copy(wet_sb[:, ec, :], pt)

        wht_sb = consts.tile([128, N_KC, D], F32)
        for kc in range(N_KC):
            pt = ps.tile([128, 512], F32, tag="ptr")
            with tc.tile_critical():
                for dp in range(N_DP):
                    nc.tensor.matmul(
                        pt[:, dp * 128:(dp + 1) * 128],
                        w_sb[:, dp, kc * 128:(kc + 1) * 128],
                        identity, is_transpose=True,
                        start=(dp == 0), stop=(dp == N_DP - 1),
                    )
            nc.scalar.copy(wht_sb[:, kc, :], pt)

        # ---- hidden^T [k, b] ----
        hidt_sb = consts.tile([128, N_KC, BP], F32)
        for kc in range(N_KC):
            pt = ps2.tile([128, 16], F32, tag="sc")
            nc.tensor.transpose(
                pt[:, 0:BP], hid_sb[:, kc * 128:(kc + 1) * 128],
                identity[0:BP, 0:BP],
            )
            nc.vector.tensor_copy(hidt_sb[:, kc, :], pt[:, 0:BP])

        # ---- b^T, v^T  [128, dp] ----
        bt_sb = consts.tile([128, N_DP], F32)
        pt = ps2.tile([128, 16], F32, tag="sc")
        nc.tensor.transpose(pt[:, 0:N_DP], b_sb4, identity[0:N_DP, 0:N_DP])
        nc.vector.tensor_copy(bt_sb, pt[:, 0:N_DP])

        vt_sb = consts.tile([128, N_DP], F16)
        pt = ps2.tile([128, 16], F32, tag="sc")
        nc.tensor.transpose(pt[:, 0:N_DP], v_sb4, identity[0:N_DP, 0:N_DP])
        nc.vector.tensor_copy(vt_sb, pt[:, 0:N_DP])

        # ---- h_projT + bias -> hbT [128, dp, b] ----
        hbt_sb = consts.tile([128, N_DP, BP], F32)
        for dp in range(N_DP):
            ph = ps2.tile([128, 16], F32, tag="sc")
            for kc in range(N_KC):
                nc.tensor.matmul(
                    ph[:, 0:BP],
                    wht_sb[:, kc, dp * 128:(dp + 1) * 128],
                    hidt_sb[:, kc, :],
                    start=(kc == 0), stop=(kc == N_KC - 1),
                )
            nc.vector.tensor_scalar_add(
                hbt_sb[:, dp, :], ph[:, 0:BP], bt_sb[:, dp:dp + 1]
            )

        # ---- main loop ----
        for bi in range(BP):
            # HWDGE f32 load (fast path), then cast f32 -> fp16 on VectorE
            # (SWDGE cast-DMA emits 2KB descriptors through one Q7 queue and
            # cannot keep the PE fed).
            enc_nat = [None] * N_SBLK
            for sblk in range(N_SBLK):
                enc32 = enc_pool.tile([128, N_ST, E], F32, tag="enc32")
                nc.sync.dma_start(
                    out=enc32,
                    in_=enc_d[bi, sblk * SBLK:(sblk + 1) * SBLK, :].rearrange(
                        "(st p) e -> p st e", p=128
                    ),
                )
                enc_tile = enc16_pool.tile([128, N_ST, E], F16, tag="enc_nat")
                enc_nat[sblk] = enc_tile
                nc.vector.tensor_copy(enc_tile, enc32)

            # transpose the whole batch: encT[e, s] for s in [0, 2048)
            enct_sb = work.tile([128, N_EC, S], F16, tag="encT")
            for sblk in range(N_SBLK):
                for ech in range(N_EC // 2):
                    pt = ps.tile([128, 1024], F16, tag="ptr")
                    with tc.tile_critical():
                        for half in range(2):
                            ec = ech * 2 + half
                            for st in range(N_ST):
                                nc.tensor.matmul(
                                    pt[:, half * 512 + st * 128:
                                       half * 512 + (st + 1) * 128],
                                    enc_nat[sblk][:, st, ec * 128:(ec + 1) * 128],
                                    identity16, is_transpose=True,
                                    start=(half == 0 and st == 0),
                                    stop=(half == 1 and st == N_ST - 1),
                                )
                    for half in range(2):
                        ec = ech * 2 + half
                        dst = enct_sb[:, ec, sblk * SBLK:(sblk + 1) * SBLK]
                        if ec % 2 == 0:
                            nc.vector.tensor_copy(dst, pt[:, half * 512:(half + 1) * 512])
                        else:
                            nc.scalar.copy(dst, pt[:, half * 512:(half + 1) * 512])

            scores_sb = small.tile([1, S], F32, tag="scores")
            for sblk in range(N_SBLK):
                energy_sb = work.tile([128, N_DP, SBLK], F16, tag="energy")
                for dp in range(N_DP):
                    pe = ps.tile([128, SBLK], F32, tag="pe")
                    for ec in range(N_EC):
                        nc.tensor.matmul(
                            pe,
                            wet_sb[:, ec, dp * 128:(dp + 1) * 128],
                            enct_sb[:, ec, sblk * SBLK:(sblk + 1) * SBLK],
                            start=(ec == 0), stop=(ec == N_EC - 1),
                        )
                    nc.scalar.activation(
                        energy_sb[:, dp, :], pe, AF.Tanh,
                        bias=hbt_sb[:, dp, bi:bi + 1], scale=1.0,
                    )

                psc = ps2.tile([1, SBLK], F32, tag="vdot")
                for dp in range(N_DP):
                    nc.tensor.matmul(
                        psc, vt_sb[:, dp:dp + 1], energy_sb[:, dp, :],
                        start=(dp == 0), stop=(dp == N_DP - 1),
                    )
                nc.scalar.copy(scores_sb[:, sblk * SBLK:(sblk + 1) * SBLK], psc)

                if debug_dumps and bi == 0 and sblk == 0:
                    nc.sync.dma_start(out=dbg_energy, in_=energy_sb)

            if debug_dumps and bi == 0:
                nc.sync.dma_start(out=dbg_enct, in_=enct_sb)

            # ---- softmax over S=2048 for this batch ----
            # Single-partition softmax on VectorE/ScalarE only: keeps the
            # PE stream free of cross-batch stalls (no transposes, no
            # broadcasts, no DRAM bounce).  ~9us/batch, fully overlapped
            # with the next batch's PE work.
            m1 = small.tile([1, 1], F32, tag="m1")
            nc.vector.reduce_max(m1, scores_sb, axis=AX.X)
            negm = small.tile([1, 1], F32, tag="negm")
            nc.vector.tensor_scalar_mul(negm, m1, -1.0)
            prob = sm1.tile([1, S], F32, tag="prob")
            nc.scalar.activation(prob, scores_sb, AF.Exp, bias=negm, scale=1.0)
            ssum = small.tile([1, 1], F32, tag="ssum")
            nc.vector.reduce_sum(ssum, prob, axis=AX.X)
            rtot = small.tile([1, 1], F32, tag="rtot")
            nc.vector.reciprocal(rtot, ssum)
            attn = sm1.tile([1, S], F32, tag="attn")
            nc.vector.tensor_scalar_mul(attn, prob, rtot)
            nc.sync.dma_start(out=out_d[bi], in_=attn)
            if debug_dumps:
                nc.sync.dma_start(out=dbg_scores[bi:bi + 1, :], in_=scores_sb)

        if debug_dumps:
            nc.sync.dma_start(out=dbg_hbt, in_=hbt_sb)
            nc.sync.dma_start(out=dbg_wet, in_=wet_sb)

    nc.compile()
    return nc


def _get_nc():
    if "nc" not in _CACHE:
        _CACHE["nc"] = _build()
    return _CACHE["nc"]


def kernel(hidden, encoder_outputs, W, b, v):
    from concourse.bass_utils import run_bass_kernel_spmd

    nc = _get_nc()
    hidden = np.ascontiguousarray(hidden, dtype=np.float32)
    encoder_outputs = np.ascontiguousarray(encoder_outputs, dtype=np.float32)
    W = np.ascontiguousarray(W, dtype=np.float32)
    b = np.ascontiguousarray(b, dtype=np.float32)
    v = np.ascontiguousarray(v, dtype=np.float32)

    in_maps = [
        {
            "hidden": hidden[c * BP:(c + 1) * BP],
            "enc": encoder_outputs[c * BP:(c + 1) * BP],
            "W": W,
            "b": b,
            "v": v,
        }
        for c in range(N_CORES)
    ]
    r = run_bass_kernel_spmd(nc, in_maps, list(range(N_CORES)))
    out = np.concatenate([r.results[c]["out"] for c in range(N_CORES)], axis=0)
    return out[:, None, :].astype(np.float32)
